# revision 18
# baseline (speedup 1.0000x reference)
"""Causal GQA self-attention (B=4, T=2048, C=2048, 16 Q heads / 8 KV heads,
hd=128) as a Bass/Tile SPMD kernel on 8 Trainium2 NeuronCores.

Sharding: core c = (batch b = c//2, head-group g = c%2). Each core handles one
batch and 8 Q heads / 4 KV heads. Wq/Wk/Wv column-sharded on the head dim, Wo
row-sharded; the host sums the two partial Wo products per batch (2-way
all-reduce done on host during the gather).

All on-device tensors live in a transposed [feature, token] layout so every
matmul contraction sits on the partition dim with no on-device transposes:
  qT/kT = [d, t], v = [t, d], scores as S^T = [k, q], output as y^T = [o, t].
Bulk matmuls run in bf16 (fp32 PSUM accumulation; ~4e-3 end-to-end rel err).

v3: Wq/Wk/Wv are host-pre-tiled and DMAed once into resident SBUF (Wo streams
per block in c2-major tiles); x is pre-tiled to 4 chunked DMAs per block. The
softmax denominator is accumulated on the DVE (acc += ex per k-tile, fp32)
with ONE ones-matmul per head instead of one per k-tile, removing 320 PE
passes. Scores/exp/AV/accumulate are all column-restricted on diagonal
(causal) k-tiles, so the GpSimd memsets are gone entirely. Projection and Wo
matmul work is chopped into small chunks by Python generators and pumped into
the exp-paced attention emission stream (attention(t) interleaves proj(t+1)
and wo(t-1) chunks), so the in-order PE queue always has dense work while the
ScalarE exp stream drains. Per-head normalization (reciprocal + GpSimd
partition-broadcast + multiply) is deferred one head so its latency hides.
"""

import sys

import ml_dtypes
import numpy as np

sys.path.insert(0, "/opt/trn_rl_repo")

import concourse.bass as bass  # noqa: E402
import concourse.mybir as mybir  # noqa: E402
import concourse.tile as tile  # noqa: E402
from concourse import bacc  # noqa: E402
from concourse.bass_utils import run_bass_kernel_spmd  # noqa: E402

# Problem shape (hardcoded per contest contract).
B = 4
T = 2048
C = 2048
HD = 128
N_HEAD = 16
N_KV_HEAD = 8
NQH = N_HEAD // 2  # q heads per core (group)
NKV = N_KV_HEAD // 2  # kv heads per core
TB = 512  # token block
NTB = T // TB
NCT = C // 128  # contraction tiles for the projections
SCALE = 1.0 / float(np.sqrt(HD))
PF = 2  # scores/exp prefetch depth in the attention pipeline
WO_START_ITER = 10  # delay wo pops until its streamed weights have landed

F32 = mybir.dt.float32
BF16 = mybir.dt.bfloat16
MULT = mybir.AluOpType.mult
ADD = mybir.AluOpType.add
EXP = mybir.ActivationFunctionType.Exp


def _rope(nc, tmpp, dst, src_psum, cosb, nsinb):
    """dst = src*cos + rot_half(src)*sin, src in [d, t] layout (d partitions).

    rot_half(x)[d] = -x[d+64] for d<64, +x[d-64] for d>=64; the sign lives in
    nsinb so both halves are plain multiplies. nsinb is the sin table rotated
    by 64 partitions (nsinb[64+i] = -sin[i], nsinb[i] = sin[64+i]) so each
    tensor_tensor has equal base partitions on its two SBUF inputs (HW rule).
    """
    t0 = tmpp.tile([HD, TB], BF16, tag="t0")
    nc.scalar.copy(t0[:], src_psum[:])
    nc.vector.tensor_mul(dst, t0[:], cosb[:])
    t2 = tmpp.tile([HD, TB], BF16, tag="t2")
    nc.vector.tensor_mul(t2[0:64, :], t0[64:128, :], nsinb[64:128, :])
    nc.vector.tensor_mul(t2[64:128, :], t0[0:64, :], nsinb[0:64, :])
    nc.vector.scalar_tensor_tensor(dst, t2[:], 1.0, dst, op0=MULT, op1=ADD)


def build_nc():
    nc = bacc.Bacc("TRN2", target_bir_lowering=False, debug=False, num_devices=8)

    # host-pre-tiled inputs (see kernel() for the exact layouts)
    xt = nc.dram_tensor("xt", [NTB, 128, NCT * TB], BF16, kind="ExternalInput")
    wq = nc.dram_tensor("wq", [128, NCT * 1024], BF16, kind="ExternalInput")
    wk = nc.dram_tensor("wk", [128, NCT * 512], BF16, kind="ExternalInput")
    wv = nc.dram_tensor("wv", [128, NCT * 512], BF16, kind="ExternalInput")
    wo = nc.dram_tensor("wo", [128, 8 * 2048], BF16, kind="ExternalInput")
    cosdt = nc.dram_tensor("cosdt", [HD, T], BF16, kind="ExternalInput")
    nsindt = nc.dram_tensor("nsindt", [HD, T], BF16, kind="ExternalInput")
    trid = nc.dram_tensor("trid", [128, 128], BF16, kind="ExternalInput")
    onescol = nc.dram_tensor("onescol", [128, 1], BF16, kind="ExternalInput")
    yT = nc.dram_tensor("yT", [C, T], BF16, kind="ExternalOutput")

    from contextlib import ExitStack

    with ExitStack() as es:
        tc = es.enter_context(tile.TileContext(nc))
        es.enter_context(nc.allow_low_precision("fp32r attention"))
        constp = es.enter_context(tc.tile_pool(name="const", bufs=1))
        strp = es.enter_context(tc.tile_pool(name="stream", bufs=2))
        perp = es.enter_context(tc.tile_pool(name="persist", bufs=1))
        xp = es.enter_context(tc.tile_pool(name="xp", bufs=1))
        wop = es.enter_context(tc.tile_pool(name="wop", bufs=8))
        qp = es.enter_context(tc.tile_pool(name="qt", bufs=10))
        outp = es.enter_context(tc.tile_pool(name="ot", bufs=8))
        tmpp = es.enter_context(tc.tile_pool(name="tmp", bufs=2))
        expp = es.enter_context(tc.tile_pool(name="exps", bufs=6))
        accp = es.enter_context(tc.tile_pool(name="acc", bufs=2))
        smallp = es.enter_context(tc.tile_pool(name="small", bufs=2))
        yp = es.enter_context(tc.tile_pool(name="ysb", bufs=2))
        projp = es.enter_context(tc.tile_pool(name="pp", bufs=3, space="PSUM"))
        spsum = es.enter_context(tc.tile_pool(name="sp", bufs=PF, space="PSUM"))
        opsum = es.enter_context(tc.tile_pool(name="op", bufs=2, space="PSUM"))
        denp = es.enter_context(tc.tile_pool(name="dp", bufs=1, space="PSUM"))
        if True:
            tri = constp.tile([128, 128], BF16, tag="tri")
            nc.sync.dma_start(tri[:], trid[:])
            ones_c = constp.tile([128, 1], BF16, tag="onesc")
            nc.sync.dma_start(ones_c[:], onescol[:])
            # resident weights; wk chunk-interleaved with x chunks of block 0
            # (emitted in load_block below) so the first K matmuls start ~3us
            # in instead of waiting for the whole preload
            wk_t = constp.tile([128, NCT * 512], BF16, tag="wk")
            wv_t = constp.tile([128, NCT * 512], BF16, tag="wv")
            wq_t = constp.tile([128, NCT * 1024], BF16, tag="wq")

            kT = [perp.tile([HD, T], BF16, tag=f"kT{h}", name=f"kT{h}") for h in range(NKV)]
            vT = [perp.tile([128, NKV * HD], BF16, tag=f"v{i}", name=f"v{i}") for i in range(T // 128)]

            def load_block(tb):
                tsl = slice(tb * TB, (tb + 1) * TB)
                xblk = xp.tile([128, NCT * TB], BF16, tag="xblk", name=f"xblk{tb}")
                nch = 8 if tb == 0 else 4
                qtr = (NCT * TB) // nch
                for ch in range(nch):
                    if tb == 0:
                        wqt = (NCT * 512) // 8
                        nc.sync.dma_start(wk_t[:, ch * wqt : (ch + 1) * wqt], wk[:, ch * wqt : (ch + 1) * wqt])
                    nc.sync.dma_start(xblk[:, ch * qtr : (ch + 1) * qtr], xt[tb][:, ch * qtr : (ch + 1) * qtr])
                cosb = strp.tile([HD, TB], BF16, tag="cosb", name=f"cosb{tb}")
                nc.sync.dma_start(cosb[:], cosdt[:, tsl])
                nsinb = strp.tile([HD, TB], BF16, tag="nsinb", name=f"nsinb{tb}")
                nc.sync.dma_start(nsinb[:], nsindt[:, tsl])
                if tb == 0:
                    # chunked so the two HWDGE rings round-robin in priority
                    # order (wk/x land first, wq last)
                    for ch in range(4):
                        wqt = (NCT * 512) // 4
                        nc.sync.dma_start(wv_t[:, ch * wqt : (ch + 1) * wqt], wv[:, ch * wqt : (ch + 1) * wqt])
                    for ch in range(8):
                        wqt = (NCT * 1024) // 8
                        nc.sync.dma_start(wq_t[:, ch * wqt : (ch + 1) * wqt], wq[:, ch * wqt : (ch + 1) * wqt])
                return xblk, cosb, nsinb

            def proj_gen(tb, xblk, cosb, nsinb, qts_out):
                """Generator emitting the projections of block tb in ~8-matmul
                chunks; fills qts_out with the 8 roped q tiles."""
                tsl = slice(tb * TB, (tb + 1) * TB)
                # K projection (k^T layout [d, t]) + RoPE
                for kw in range(2):
                    kps = [projp.tile([128, TB], F32, tag="pp", name=f"kps{tb}_{kw}_{i}") for i in range(2)]
                    for ct in range(NCT):
                        wcol = ct * 512 + kw * 256
                        for i in range(2):
                            nc.tensor.matmul(
                                kps[i][:],
                                wk_t[:, wcol + i * 128 : wcol + (i + 1) * 128],
                                xblk[:, ct * TB : (ct + 1) * TB],
                                start=(ct == 0),
                                stop=(ct == NCT - 1),
                            )
                        if ct % 4 == 3 and ct < NCT - 1:
                            yield
                    for i in range(2):
                        _rope(nc, tmpp, kT[kw * 2 + i][:, tsl], kps[i], cosb, nsinb)
                    yield

                # V projection in [t, d] layout; psum evacuated on ScalarE so
                # the DVE rope stream can't back up the projp psum ring
                for vw in range(2):
                    vps = [projp.tile([128, NKV * HD], F32, tag="pp", name=f"vps{tb}_{vw}_{i}") for i in range(2)]
                    for ct in range(NCT):
                        for i in range(2):
                            tcol = ct * TB + (vw * 2 + i) * 128
                            nc.tensor.matmul(
                                vps[i][:],
                                xblk[:, tcol : tcol + 128],
                                wv_t[:, ct * 512 : (ct + 1) * 512],
                                start=(ct == 0),
                                stop=(ct == NCT - 1),
                            )
                        if ct % 4 == 3 and ct < NCT - 1:
                            yield
                    for i in range(2):
                        nc.scalar.copy(vT[4 * tb + vw * 2 + i][:], vps[i][:])
                    yield

                # Q projection (q^T layout) + RoPE, four waves of 2
                for wave in range(4):
                    qps = [projp.tile([128, TB], F32, tag="pp", name=f"qps{tb}_{wave}_{i}") for i in range(2)]
                    for ct in range(NCT):
                        wcol = ct * 1024 + wave * 256
                        for o in range(2):
                            nc.tensor.matmul(
                                qps[o][:],
                                wq_t[:, wcol + o * 128 : wcol + (o + 1) * 128],
                                xblk[:, ct * TB : (ct + 1) * TB],
                                start=(ct == 0),
                                stop=(ct == NCT - 1),
                            )
                        if ct % 4 == 3 and ct < NCT - 1:
                            yield
                    for o in range(2):
                        qt = qp.tile([HD, TB], BF16, tag="qt", name=f"qt{tb}_{wave}_{o}")
                        _rope(nc, tmpp, qt[:], qps[o], cosb, nsinb)
                        qts_out.append(qt)
                    yield

            def wo_gen(tb, outs, dve_evac=False):
                """Generator emitting wo @ outs(tb) in half-c2 chunks. Weights
                stream per block as c2-major [128, 2048] tiles on the scalar
                HWDGE queue (so they don't sit behind xblk on the sync queue).
                dve_evac moves the psum->sbuf copies to the DVE for phases
                where ScalarE is exp-saturated."""
                tsl = slice(tb * TB, (tb + 1) * TB)
                tiles = {}

                def load(c2):
                    t_ = wop.tile([128, 2048], BF16, tag="wo", name=f"wo{tb}_{c2}")
                    nc.scalar.dma_start(t_[:], wo[:, c2 * 2048 : (c2 + 1) * 2048])
                    tiles[c2] = t_

                for c2 in range(8):
                    load(c2)
                for c2 in range(8):
                    yps = [projp.tile([128, TB], F32, tag="pp", name=f"yps{tb}_{c2}_{i}") for i in range(2)]
                    wt = tiles.pop(c2)
                    for jh in range(NQH):
                        wcol = jh * 256
                        for o in range(2):
                            nc.tensor.matmul(
                                yps[o][:],
                                wt[:, wcol + o * 128 : wcol + (o + 1) * 128],
                                outs[jh][:],
                                start=(jh == 0),
                                stop=(jh == NQH - 1),
                            )
                        if jh == 3:
                            yield
                    for o in range(2):
                        ysb = yp.tile([128, TB], BF16, tag="ysb")
                        if dve_evac:
                            nc.vector.tensor_copy(ysb[:], yps[o][:])
                        else:
                            nc.scalar.copy(ysb[:], yps[o][:])
                        og = c2 * 2 + o
                        nc.sync.dma_start(yT[og * 128 : (og + 1) * 128, tsl], ysb[:])
                    yield

            def attention_block(tb, qts, wgen, pgen, n_wo, n_proj):
                """Attention of block tb, pumping chunks from wo(tb-1) and
                proj(tb+1) generators into the PE stream between iterations."""
                ktmax = 4 * tb + 4
                iters_total = NQH * ktmax
                total_chunks = n_wo + n_proj
                rate = total_chunks / iters_total
                state = {"q": 0.0, "it": 0, "w": wgen, "p": pgen, "wi": 0}

                def pop_one():
                    # prefer wo once past WO_START_ITER (2 proj : 1 wo), else proj
                    order = []
                    if state["it"] >= WO_START_ITER and state["w"] is not None:
                        if state["wi"] % 3 == 0:
                            order = ["w", "p"]
                        else:
                            order = ["p", "w"]
                        state["wi"] += 1
                    else:
                        order = ["p", "w"] if state["it"] >= WO_START_ITER else ["p"]
                    for k in order:
                        g = state[k]
                        if g is None:
                            continue
                        try:
                            next(g)
                            return True
                        except StopIteration:
                            state[k] = None
                    return False

                def pump():
                    state["it"] += 1
                    state["q"] += rate
                    while state["q"] >= 1.0:
                        state["q"] -= 1.0
                        if not pop_one():
                            state["q"] = 0.0
                            break

                def emit_s(h, kt):
                    # scores + exp for one k-tile; on diagonal tiles only
                    # the live q columns [q0:TB] are computed/consumed
                    hv = h // 2
                    m = kt - 4 * tb
                    q0 = 128 * m if m > 0 else 0
                    sps = spsum.tile([128, TB], F32, tag="sp")
                    nc.tensor.matmul(
                        sps[:, q0:TB],
                        kT[hv][:, kt * 128 : (kt + 1) * 128],
                        qts[h][:, q0:TB],
                        start=True,
                        stop=True,
                    )
                    ex = expp.tile([128, TB], BF16, tag="exps")
                    nc.scalar.activation(ex[:, q0:TB], sps[:, q0:TB], EXP, scale=SCALE)
                    if m >= 0:
                        nc.vector.tensor_mul(
                            ex[:, q0 : q0 + 128],
                            ex[:, q0 : q0 + 128],
                            tri[:],
                        )
                    return ex, q0

                # flattened (h, kt) stream: scores/exp run PF iterations ahead
                # ACROSS head boundaries so a new head's first AV never waits
                # on a cold exp
                from collections import deque

                pairs = [(h, kt) for h in range(NQH) for kt in range(ktmax)]
                exq = deque()
                ei = 0
                while ei < min(PF, len(pairs)):
                    exq.append(emit_s(*pairs[ei]))
                    ei += 1
                outs = [None] * NQH
                pending = None  # (h, ops_, bcs) awaiting its normalization mul
                ops_ = None
                acc = None
                for h, kt in pairs:
                    if ei < len(pairs):
                        exq.append(emit_s(*pairs[ei]))
                        ei += 1
                    ex, q0 = exq.popleft()
                    if kt == 0:
                        ops_ = opsum.tile([HD, TB], F32, tag="op", name=f"aop{tb}_{h}")
                        acc = accp.tile([128, TB], BF16, tag="acc", name=f"acc{tb}_{h}")
                        nc.vector.tensor_copy(acc[:], ex[:])
                    else:
                        # denominator partial sums accumulate on the DVE
                        nc.vector.tensor_add(acc[:, q0:TB], acc[:, q0:TB], ex[:, q0:TB])
                    nc.tensor.matmul(
                        ops_[:, q0:TB],
                        vT[kt][:, (h // 2) * 128 : (h // 2 + 1) * 128],
                        ex[:, q0:TB],
                        start=(kt == 0),
                        stop=(kt == ktmax - 1),
                    )
                    if kt == ktmax // 2 and pending is not None:
                        # deferred normalization of the previous head, emitted
                        # mid-head so the GpSimd broadcast latency hides and the
                        # ops_ psum ring (2) is free before the next head starts
                        ph, pops_, pbcs = pending
                        ot = outp.tile([HD, TB], BF16, tag="ot")
                        nc.vector.tensor_mul(ot[:], pops_[:], pbcs[:])
                        outs[ph] = ot
                        pending = None
                    if kt == ktmax - 1:
                        # one ones-matmul per head closes the denominator
                        den = denp.tile([1, TB], F32, tag="dp", name=f"den{tb}_{h}")
                        nc.tensor.matmul(den[:], ones_c[:], acc[:], start=True, stop=True)
                        rec = smallp.tile([1, TB], F32, tag="rec")
                        nc.vector.reciprocal_approx_fast(rec[:], den[:])
                        bcs = smallp.tile([128, TB], F32, tag="bcs")
                        nc.gpsimd.partition_broadcast(bcs[:], rec[0:1, :])
                        pending = (h, ops_, bcs)
                    pump()
                ph, pops_, pbcs = pending
                ot = outp.tile([HD, TB], BF16, tag="ot")
                nc.vector.tensor_mul(ot[:], pops_[:], pbcs[:])
                outs[ph] = ot
                # drain any remaining interleave chunks
                while pop_one():
                    pass
                return outs

            # Pipeline: P0 | A0+P1 | A1+P2+W0 | A2+P3+W1 | A3+W2 | W3
            qts_all = {}
            xblk, cosb, nsinb = load_block(0)
            qts_all[0] = []
            for _ in proj_gen(0, xblk, cosb, nsinb, qts_all[0]):
                pass
            outs_prev = None
            for t in range(NTB):
                pgen = None
                n_proj = 0
                if t + 1 < NTB:
                    xblk, cosb, nsinb = load_block(t + 1)
                    qts_all[t + 1] = []
                    pgen = proj_gen(t + 1, xblk, cosb, nsinb, qts_all[t + 1])
                    n_proj = 32
                wgen = None
                n_wo = 0
                if outs_prev is not None:
                    wgen = wo_gen(t - 1, outs_prev, dve_evac=(t >= 3))
                    n_wo = 16
                outs_prev = attention_block(t, qts_all[t], wgen, pgen, n_wo, n_proj)
            for _ in wo_gen(NTB - 1, outs_prev):
                pass

    nc.compile()
    return nc


def _host_consts():
    inv_freq = 1.0 / (10000.0 ** (np.arange(0, HD, 2, dtype=np.float32) / HD))
    t = np.arange(T, dtype=np.float32)
    freqs = np.outer(t, inv_freq)  # [T, HD/2]
    freqs = np.repeat(freqs, 2, axis=-1)  # [T, HD]
    cos = np.cos(freqs).astype(np.float32).T.copy()  # [HD, T]
    sin = np.sin(freqs).astype(np.float32).T.copy()
    # rotated-by-64 signed sin table: row d holds the multiplier that pairs
    # with x[(d+64)%128]; rows 64..127 carry -sin[0:64], rows 0..63 +sin[64:128]
    nsin = np.empty_like(sin)
    nsin[0:64, :] = sin[64:128, :]
    nsin[64:128, :] = -sin[0:64, :]

    kp = np.arange(128)[:, None]
    qf = np.arange(128)[None, :]
    tri = (kp <= qf).astype(ml_dtypes.bfloat16)

    return {
        "cosdt": np.ascontiguousarray(cos).astype(ml_dtypes.bfloat16),
        "nsindt": np.ascontiguousarray(nsin).astype(ml_dtypes.bfloat16),
        "trid": tri,
        "onescol": np.ones((128, 1), dtype=ml_dtypes.bfloat16),
    }


_NC_CACHE = None


def _get_nc():
    global _NC_CACHE
    if _NC_CACHE is None:
        _NC_CACHE = build_nc()
    return _NC_CACHE


def kernel(x, Wq, Wk, Wv, Wo, _trace=False):
    x = np.asarray(x, dtype=np.float32)
    Wq = np.asarray(Wq, dtype=np.float32)
    Wk = np.asarray(Wk, dtype=np.float32)
    Wv = np.asarray(Wv, dtype=np.float32)
    Wo = np.asarray(Wo, dtype=np.float32)

    nc = _get_nc()
    consts = _host_consts()

    bf = ml_dtypes.bfloat16
    # x pre-tiled per batch: xt[tb][p][ct*TB + t] = x[b, tb*TB + t, ct*128 + p]
    xts = [
        np.ascontiguousarray(
            x[b].astype(bf).reshape(NTB, TB, NCT, 128).transpose(0, 3, 2, 1).reshape(NTB, 128, NCT * TB)
        )
        for b in range(B)
    ]

    def _tile_w(A, w):  # A: [dout, C] -> [128, NCT*w], w = dout per group
        # out[p, ct*w + j] = A[j, ct*128 + p]
        return np.ascontiguousarray(A.T.reshape(NCT, 128, w).transpose(1, 0, 2).reshape(128, NCT * w)).astype(bf)

    wqs = [_tile_w(Wq[1024 * g : 1024 * (g + 1), :], 1024) for g in range(2)]
    wks = [_tile_w(Wk[512 * g : 512 * (g + 1), :], 512) for g in range(2)]
    wvs = [_tile_w(Wv[512 * g : 512 * (g + 1), :], 512) for g in range(2)]
    # c2-major wo: wo[p, c2*2048 + jh*256 + jo] = Wo[c2*256 + jo, 1024g + jh*128 + p]
    wos = [
        np.ascontiguousarray(
            Wo[:, 1024 * g : 1024 * (g + 1)]
            .T.reshape(NQH, 128, 8, 256)
            .transpose(1, 2, 0, 3)
            .reshape(128, 8 * 2048)
        ).astype(bf)
        for g in range(2)
    ]

    in_maps = []
    for c in range(8):
        b, g = c // 2, c % 2
        im = {
            "xt": xts[b],
            "wq": wqs[g],
            "wk": wks[g],
            "wv": wvs[g],
            "wo": wos[g],
        }
        im.update(consts)
        in_maps.append(im)

    res = run_bass_kernel_spmd(nc, in_maps, core_ids=list(range(8)), trace=_trace)

    y = np.empty((B, T, C), dtype=np.float32)
    for b in range(B):
        y[b] = (
            res.results[2 * b]["yT"].astype(np.float32) + res.results[2 * b + 1]["yT"].astype(np.float32)
        ).T
    if _trace:
        return y, res
    return y


# revision 19
# speedup vs baseline: 1.0120x; 1.0120x over previous
"""Causal GQA self-attention (B=4, T=2048, C=2048, 16 Q heads / 8 KV heads,
hd=128) as a Bass/Tile SPMD kernel on 8 Trainium2 NeuronCores.

Sharding: core c = (batch b = c//2, head-group g = c%2). Each core handles one
batch and 8 Q heads / 4 KV heads. Wq/Wk/Wv column-sharded on the head dim, Wo
row-sharded; the host sums the two partial Wo products per batch (2-way
all-reduce done on host during the gather).

All on-device tensors live in a transposed [feature, token] layout so every
matmul contraction sits on the partition dim with no on-device transposes:
  qT/kT = [d, t], v = [t, d], scores as S^T = [k, q], output as y^T = [o, t].
Bulk matmuls run in bf16 (fp32 PSUM accumulation; ~4e-3 end-to-end rel err).

v3: Wq/Wk/Wv are host-pre-tiled and DMAed once into resident SBUF (Wo streams
per block in c2-major tiles); x is pre-tiled to 4 chunked DMAs per block. The
softmax denominator is accumulated on the DVE (acc += ex per k-tile, fp32)
with ONE ones-matmul per head instead of one per k-tile, removing 320 PE
passes. Scores/exp/AV/accumulate are all column-restricted on diagonal
(causal) k-tiles, so the GpSimd memsets are gone entirely. Projection and Wo
matmul work is chopped into small chunks by Python generators and pumped into
the exp-paced attention emission stream (attention(t) interleaves proj(t+1)
and wo(t-1) chunks), so the in-order PE queue always has dense work while the
ScalarE exp stream drains. Per-head normalization (reciprocal + GpSimd
partition-broadcast + multiply) is deferred one head so its latency hides.
"""

import sys

import ml_dtypes
import numpy as np

sys.path.insert(0, "/opt/trn_rl_repo")

import concourse.bass as bass  # noqa: E402
import concourse.mybir as mybir  # noqa: E402
import concourse.tile as tile  # noqa: E402
from concourse import bacc  # noqa: E402
from concourse.bass_utils import run_bass_kernel_spmd  # noqa: E402

# Problem shape (hardcoded per contest contract).
B = 4
T = 2048
C = 2048
HD = 128
N_HEAD = 16
N_KV_HEAD = 8
NQH = N_HEAD // 2  # q heads per core (group)
NKV = N_KV_HEAD // 2  # kv heads per core
TB = 512  # token block
NTB = T // TB
NCT = C // 128  # contraction tiles for the projections
SCALE = 1.0 / float(np.sqrt(HD))
PF = 2  # scores/exp prefetch depth in the attention pipeline
WO_START_ITER = 6  # delay wo pops until its streamed weights have landed

F32 = mybir.dt.float32
BF16 = mybir.dt.bfloat16
MULT = mybir.AluOpType.mult
ADD = mybir.AluOpType.add
EXP = mybir.ActivationFunctionType.Exp


def _rope(nc, tmpp, dst, src_psum, cosb, nsinb):
    """dst = src*cos + rot_half(src)*sin, src in [d, t] layout (d partitions).

    rot_half(x)[d] = -x[d+64] for d<64, +x[d-64] for d>=64; the sign lives in
    nsinb so both halves are plain multiplies. nsinb is the sin table rotated
    by 64 partitions (nsinb[64+i] = -sin[i], nsinb[i] = sin[64+i]) so each
    tensor_tensor has equal base partitions on its two SBUF inputs (HW rule).
    """
    t0 = tmpp.tile([HD, TB], BF16, tag="t0")
    nc.scalar.copy(t0[:], src_psum[:])
    nc.vector.tensor_mul(dst, t0[:], cosb[:])
    t2 = tmpp.tile([HD, TB], BF16, tag="t2")
    nc.vector.tensor_mul(t2[0:64, :], t0[64:128, :], nsinb[64:128, :])
    nc.vector.tensor_mul(t2[64:128, :], t0[0:64, :], nsinb[0:64, :])
    nc.vector.scalar_tensor_tensor(dst, t2[:], 1.0, dst, op0=MULT, op1=ADD)


def build_nc():
    nc = bacc.Bacc("TRN2", target_bir_lowering=False, debug=False, num_devices=8)

    # host-pre-tiled inputs (see kernel() for the exact layouts)
    xt = nc.dram_tensor("xt", [NTB, 128, NCT * TB], BF16, kind="ExternalInput")
    wq = nc.dram_tensor("wq", [128, NCT * 1024], BF16, kind="ExternalInput")
    wk = nc.dram_tensor("wk", [128, NCT * 512], BF16, kind="ExternalInput")
    wv = nc.dram_tensor("wv", [128, NCT * 512], BF16, kind="ExternalInput")
    wo = nc.dram_tensor("wo", [128, 8 * 2048], BF16, kind="ExternalInput")
    cosdt = nc.dram_tensor("cosdt", [HD, T], BF16, kind="ExternalInput")
    nsindt = nc.dram_tensor("nsindt", [HD, T], BF16, kind="ExternalInput")
    trid = nc.dram_tensor("trid", [128, 128], BF16, kind="ExternalInput")
    onescol = nc.dram_tensor("onescol", [128, 1], BF16, kind="ExternalInput")
    yT = nc.dram_tensor("yT", [C, T], BF16, kind="ExternalOutput")

    from contextlib import ExitStack

    with ExitStack() as es:
        tc = es.enter_context(tile.TileContext(nc))
        es.enter_context(nc.allow_low_precision("fp32r attention"))
        constp = es.enter_context(tc.tile_pool(name="const", bufs=1))
        strp = es.enter_context(tc.tile_pool(name="stream", bufs=2))
        perp = es.enter_context(tc.tile_pool(name="persist", bufs=1))
        xp = es.enter_context(tc.tile_pool(name="xp", bufs=1))
        wop = es.enter_context(tc.tile_pool(name="wop", bufs=8))
        qp = es.enter_context(tc.tile_pool(name="qt", bufs=10))
        outp = es.enter_context(tc.tile_pool(name="ot", bufs=8))
        tmpp = es.enter_context(tc.tile_pool(name="tmp", bufs=2))
        expp = es.enter_context(tc.tile_pool(name="exps", bufs=8))
        accp = es.enter_context(tc.tile_pool(name="acc", bufs=2))
        smallp = es.enter_context(tc.tile_pool(name="small", bufs=3))
        yp = es.enter_context(tc.tile_pool(name="ysb", bufs=3))
        projp = es.enter_context(tc.tile_pool(name="pp", bufs=3, space="PSUM"))
        spsum = es.enter_context(tc.tile_pool(name="sp", bufs=PF, space="PSUM"))
        opsum = es.enter_context(tc.tile_pool(name="op", bufs=2, space="PSUM"))
        denp = es.enter_context(tc.tile_pool(name="dp", bufs=1, space="PSUM"))
        if True:
            tri = constp.tile([128, 128], BF16, tag="tri")
            nc.sync.dma_start(tri[:], trid[:])
            ones_c = constp.tile([128, 1], BF16, tag="onesc")
            nc.sync.dma_start(ones_c[:], onescol[:])
            # resident weights; wk chunk-interleaved with x chunks of block 0
            # (emitted in load_block below) so the first K matmuls start ~3us
            # in instead of waiting for the whole preload
            wk_t = constp.tile([128, NCT * 512], BF16, tag="wk")
            wv_t = constp.tile([128, NCT * 512], BF16, tag="wv")
            wq_t = constp.tile([128, NCT * 1024], BF16, tag="wq")

            kT = [perp.tile([HD, T], BF16, tag=f"kT{h}", name=f"kT{h}") for h in range(NKV)]
            vT = [perp.tile([128, NKV * HD], BF16, tag=f"v{i}", name=f"v{i}") for i in range(T // 128)]

            def load_block(tb):
                tsl = slice(tb * TB, (tb + 1) * TB)
                xblk = xp.tile([128, NCT * TB], BF16, tag="xblk", name=f"xblk{tb}")
                nch = 8 if tb == 0 else 4
                qtr = (NCT * TB) // nch
                for ch in range(nch):
                    if tb == 0:
                        wqt = (NCT * 512) // 8
                        nc.sync.dma_start(wk_t[:, ch * wqt : (ch + 1) * wqt], wk[:, ch * wqt : (ch + 1) * wqt])
                    nc.sync.dma_start(xblk[:, ch * qtr : (ch + 1) * qtr], xt[tb][:, ch * qtr : (ch + 1) * qtr])
                cosb = strp.tile([HD, TB], BF16, tag="cosb", name=f"cosb{tb}")
                nc.sync.dma_start(cosb[:], cosdt[:, tsl])
                nsinb = strp.tile([HD, TB], BF16, tag="nsinb", name=f"nsinb{tb}")
                nc.sync.dma_start(nsinb[:], nsindt[:, tsl])
                if tb == 0:
                    # chunked so the two HWDGE rings round-robin in priority
                    # order (wk/x land first, wq last)
                    for ch in range(4):
                        wqt = (NCT * 512) // 4
                        nc.sync.dma_start(wv_t[:, ch * wqt : (ch + 1) * wqt], wv[:, ch * wqt : (ch + 1) * wqt])
                    for ch in range(8):
                        wqt = (NCT * 1024) // 8
                        nc.sync.dma_start(wq_t[:, ch * wqt : (ch + 1) * wqt], wq[:, ch * wqt : (ch + 1) * wqt])
                return xblk, cosb, nsinb

            def proj_gen(tb, xblk, cosb, nsinb, qts_out):
                """Generator emitting the projections of block tb in ~8-matmul
                chunks; fills qts_out with the 8 roped q tiles."""
                tsl = slice(tb * TB, (tb + 1) * TB)
                # K projection (k^T layout [d, t]) + RoPE
                for kw in range(2):
                    kps = [projp.tile([128, TB], F32, tag="pp", name=f"kps{tb}_{kw}_{i}") for i in range(2)]
                    for ct in range(NCT):
                        wcol = ct * 512 + kw * 256
                        for i in range(2):
                            nc.tensor.matmul(
                                kps[i][:],
                                wk_t[:, wcol + i * 128 : wcol + (i + 1) * 128],
                                xblk[:, ct * TB : (ct + 1) * TB],
                                start=(ct == 0),
                                stop=(ct == NCT - 1),
                            )
                        if ct % 4 == 3 and ct < NCT - 1:
                            yield
                    for i in range(2):
                        _rope(nc, tmpp, kT[kw * 2 + i][:, tsl], kps[i], cosb, nsinb)
                    yield

                # V projection in [t, d] layout; psum evacuated on ScalarE so
                # the DVE rope stream can't back up the projp psum ring
                for vw in range(2):
                    vps = [projp.tile([128, NKV * HD], F32, tag="pp", name=f"vps{tb}_{vw}_{i}") for i in range(2)]
                    for ct in range(NCT):
                        for i in range(2):
                            tcol = ct * TB + (vw * 2 + i) * 128
                            nc.tensor.matmul(
                                vps[i][:],
                                xblk[:, tcol : tcol + 128],
                                wv_t[:, ct * 512 : (ct + 1) * 512],
                                start=(ct == 0),
                                stop=(ct == NCT - 1),
                            )
                        if ct % 4 == 3 and ct < NCT - 1:
                            yield
                    for i in range(2):
                        nc.scalar.copy(vT[4 * tb + vw * 2 + i][:], vps[i][:])
                    yield

                # Q projection (q^T layout) + RoPE, four waves of 2
                for wave in range(4):
                    qps = [projp.tile([128, TB], F32, tag="pp", name=f"qps{tb}_{wave}_{i}") for i in range(2)]
                    for ct in range(NCT):
                        wcol = ct * 1024 + wave * 256
                        for o in range(2):
                            nc.tensor.matmul(
                                qps[o][:],
                                wq_t[:, wcol + o * 128 : wcol + (o + 1) * 128],
                                xblk[:, ct * TB : (ct + 1) * TB],
                                start=(ct == 0),
                                stop=(ct == NCT - 1),
                            )
                        if ct % 4 == 3 and ct < NCT - 1:
                            yield
                    for o in range(2):
                        qt = qp.tile([HD, TB], BF16, tag="qt", name=f"qt{tb}_{wave}_{o}")
                        _rope(nc, tmpp, qt[:], qps[o], cosb, nsinb)
                        qts_out.append(qt)
                    yield

            def wo_gen(tb, outs, dve_evac=False):
                """Generator emitting wo @ outs(tb) in half-c2 chunks. Weights
                stream per block as c2-major [128, 2048] tiles on the scalar
                HWDGE queue (so they don't sit behind xblk on the sync queue).
                dve_evac moves the psum->sbuf copies to the DVE for phases
                where ScalarE is exp-saturated."""
                tsl = slice(tb * TB, (tb + 1) * TB)
                tiles = {}

                def load(c2):
                    t_ = wop.tile([128, 2048], BF16, tag="wo", name=f"wo{tb}_{c2}")
                    nc.scalar.dma_start(t_[:], wo[:, c2 * 2048 : (c2 + 1) * 2048])
                    tiles[c2] = t_

                for c2 in range(8):
                    load(c2)
                for c2 in range(8):
                    yps = [projp.tile([128, TB], F32, tag="pp", name=f"yps{tb}_{c2}_{i}") for i in range(2)]
                    wt = tiles.pop(c2)
                    for jh in range(NQH):
                        wcol = jh * 256
                        for o in range(2):
                            nc.tensor.matmul(
                                yps[o][:],
                                wt[:, wcol + o * 128 : wcol + (o + 1) * 128],
                                outs[jh][:],
                                start=(jh == 0),
                                stop=(jh == NQH - 1),
                            )
                        if jh == 3:
                            yield
                    for o in range(2):
                        ysb = yp.tile([128, TB], BF16, tag="ysb")
                        if dve_evac:
                            nc.vector.tensor_copy(ysb[:], yps[o][:])
                        else:
                            nc.scalar.copy(ysb[:], yps[o][:])
                        og = c2 * 2 + o
                        nc.sync.dma_start(yT[og * 128 : (og + 1) * 128, tsl], ysb[:])
                    yield

            def attention_block(tb, qts, wgen, pgen, n_wo, n_proj):
                """Attention of block tb, pumping chunks from wo(tb-1) and
                proj(tb+1) generators into the PE stream between iterations."""
                ktmax = 4 * tb + 4
                iters_total = NQH * ktmax
                total_chunks = n_wo + n_proj
                rate = total_chunks / iters_total
                state = {"q": 0.0, "it": 0, "w": wgen, "p": pgen, "wi": 0}

                def pop_one():
                    # prefer wo once past WO_START_ITER (2 proj : 1 wo), else proj
                    order = []
                    if state["it"] >= WO_START_ITER and state["w"] is not None:
                        if state["wi"] % 3 == 0:
                            order = ["w", "p"]
                        else:
                            order = ["p", "w"]
                        state["wi"] += 1
                    else:
                        order = ["p", "w"] if state["it"] >= WO_START_ITER else ["p"]
                    for k in order:
                        g = state[k]
                        if g is None:
                            continue
                        try:
                            next(g)
                            return True
                        except StopIteration:
                            state[k] = None
                    return False

                def pump():
                    state["it"] += 1
                    state["q"] += rate
                    while state["q"] >= 1.0:
                        state["q"] -= 1.0
                        if not pop_one():
                            state["q"] = 0.0
                            break

                def emit_s(h, kt):
                    # scores + exp for one k-tile; on diagonal tiles only
                    # the live q columns [q0:TB] are computed/consumed
                    hv = h // 2
                    m = kt - 4 * tb
                    q0 = 128 * m if m > 0 else 0
                    sps = spsum.tile([128, TB], F32, tag="sp")
                    nc.tensor.matmul(
                        sps[:, q0:TB],
                        kT[hv][:, kt * 128 : (kt + 1) * 128],
                        qts[h][:, q0:TB],
                        start=True,
                        stop=True,
                    )
                    ex = expp.tile([128, TB], BF16, tag="exps")
                    nc.scalar.activation(ex[:, q0:TB], sps[:, q0:TB], EXP, scale=SCALE)
                    if m >= 0:
                        nc.vector.tensor_mul(
                            ex[:, q0 : q0 + 128],
                            ex[:, q0 : q0 + 128],
                            tri[:],
                        )
                    return ex, q0

                # flattened (h, kt) stream: scores/exp run PF iterations ahead
                # ACROSS head boundaries so a new head's first AV never waits
                # on a cold exp
                from collections import deque

                pairs = [(h, kt) for h in range(NQH) for kt in range(ktmax)]
                exq = deque()
                ei = 0
                while ei < min(PF, len(pairs)):
                    exq.append(emit_s(*pairs[ei]))
                    ei += 1
                outs = [None] * NQH
                pending = None  # (h, ops_, bcs) awaiting its normalization mul
                ops_ = None
                acc = None
                for h, kt in pairs:
                    if ei < len(pairs):
                        exq.append(emit_s(*pairs[ei]))
                        ei += 1
                    ex, q0 = exq.popleft()
                    if kt == 0:
                        ops_ = opsum.tile([HD, TB], F32, tag="op", name=f"aop{tb}_{h}")
                        acc = accp.tile([128, TB], BF16, tag="acc", name=f"acc{tb}_{h}")
                        nc.vector.tensor_copy(acc[:], ex[:])
                    else:
                        # denominator partial sums accumulate on the DVE
                        nc.vector.tensor_add(acc[:, q0:TB], acc[:, q0:TB], ex[:, q0:TB])
                    nc.tensor.matmul(
                        ops_[:, q0:TB],
                        vT[kt][:, (h // 2) * 128 : (h // 2 + 1) * 128],
                        ex[:, q0:TB],
                        start=(kt == 0),
                        stop=(kt == ktmax - 1),
                    )
                    if kt == ktmax // 2 and pending is not None:
                        # deferred normalization of the previous head, emitted
                        # mid-head so the GpSimd broadcast latency hides and the
                        # ops_ psum ring (2) is free before the next head starts
                        ph, pops_, pbcs = pending
                        ot = outp.tile([HD, TB], BF16, tag="ot")
                        nc.vector.tensor_mul(ot[:], pops_[:], pbcs[:])
                        outs[ph] = ot
                        pending = None
                    if kt == ktmax - 1:
                        # one ones-matmul per head closes the denominator
                        den = denp.tile([1, TB], F32, tag="dp", name=f"den{tb}_{h}")
                        nc.tensor.matmul(den[:], ones_c[:], acc[:], start=True, stop=True)
                        rec = smallp.tile([1, TB], F32, tag="rec")
                        nc.vector.reciprocal_approx_fast(rec[:], den[:])
                        bcs = smallp.tile([128, TB], F32, tag="bcs")
                        nc.gpsimd.partition_broadcast(bcs[:], rec[0:1, :])
                        pending = (h, ops_, bcs)
                    pump()
                ph, pops_, pbcs = pending
                ot = outp.tile([HD, TB], BF16, tag="ot")
                nc.vector.tensor_mul(ot[:], pops_[:], pbcs[:])
                outs[ph] = ot
                # drain any remaining interleave chunks
                while pop_one():
                    pass
                return outs

            # Pipeline: P0 | A0+P1 | A1+P2+W0 | A2+P3+W1 | A3+W2 | W3
            qts_all = {}
            xblk, cosb, nsinb = load_block(0)
            qts_all[0] = []
            for _ in proj_gen(0, xblk, cosb, nsinb, qts_all[0]):
                pass
            outs_prev = None
            for t in range(NTB):
                pgen = None
                n_proj = 0
                if t + 1 < NTB:
                    xblk, cosb, nsinb = load_block(t + 1)
                    qts_all[t + 1] = []
                    pgen = proj_gen(t + 1, xblk, cosb, nsinb, qts_all[t + 1])
                    n_proj = 32
                wgen = None
                n_wo = 0
                if outs_prev is not None:
                    wgen = wo_gen(t - 1, outs_prev, dve_evac=(t >= 3))
                    n_wo = 16
                outs_prev = attention_block(t, qts_all[t], wgen, pgen, n_wo, n_proj)
            for _ in wo_gen(NTB - 1, outs_prev):
                pass

    nc.compile()
    return nc


def _host_consts():
    inv_freq = 1.0 / (10000.0 ** (np.arange(0, HD, 2, dtype=np.float32) / HD))
    t = np.arange(T, dtype=np.float32)
    freqs = np.outer(t, inv_freq)  # [T, HD/2]
    freqs = np.repeat(freqs, 2, axis=-1)  # [T, HD]
    cos = np.cos(freqs).astype(np.float32).T.copy()  # [HD, T]
    sin = np.sin(freqs).astype(np.float32).T.copy()
    # rotated-by-64 signed sin table: row d holds the multiplier that pairs
    # with x[(d+64)%128]; rows 64..127 carry -sin[0:64], rows 0..63 +sin[64:128]
    nsin = np.empty_like(sin)
    nsin[0:64, :] = sin[64:128, :]
    nsin[64:128, :] = -sin[0:64, :]

    kp = np.arange(128)[:, None]
    qf = np.arange(128)[None, :]
    tri = (kp <= qf).astype(ml_dtypes.bfloat16)

    return {
        "cosdt": np.ascontiguousarray(cos).astype(ml_dtypes.bfloat16),
        "nsindt": np.ascontiguousarray(nsin).astype(ml_dtypes.bfloat16),
        "trid": tri,
        "onescol": np.ones((128, 1), dtype=ml_dtypes.bfloat16),
    }


_NC_CACHE = None


def _get_nc():
    global _NC_CACHE
    if _NC_CACHE is None:
        _NC_CACHE = build_nc()
    return _NC_CACHE


def kernel(x, Wq, Wk, Wv, Wo, _trace=False):
    x = np.asarray(x, dtype=np.float32)
    Wq = np.asarray(Wq, dtype=np.float32)
    Wk = np.asarray(Wk, dtype=np.float32)
    Wv = np.asarray(Wv, dtype=np.float32)
    Wo = np.asarray(Wo, dtype=np.float32)

    nc = _get_nc()
    consts = _host_consts()

    bf = ml_dtypes.bfloat16
    # x pre-tiled per batch: xt[tb][p][ct*TB + t] = x[b, tb*TB + t, ct*128 + p]
    xts = [
        np.ascontiguousarray(
            x[b].astype(bf).reshape(NTB, TB, NCT, 128).transpose(0, 3, 2, 1).reshape(NTB, 128, NCT * TB)
        )
        for b in range(B)
    ]

    def _tile_w(A, w):  # A: [dout, C] -> [128, NCT*w], w = dout per group
        # out[p, ct*w + j] = A[j, ct*128 + p]
        return np.ascontiguousarray(A.T.reshape(NCT, 128, w).transpose(1, 0, 2).reshape(128, NCT * w)).astype(bf)

    wqs = [_tile_w(Wq[1024 * g : 1024 * (g + 1), :], 1024) for g in range(2)]
    wks = [_tile_w(Wk[512 * g : 512 * (g + 1), :], 512) for g in range(2)]
    wvs = [_tile_w(Wv[512 * g : 512 * (g + 1), :], 512) for g in range(2)]
    # c2-major wo: wo[p, c2*2048 + jh*256 + jo] = Wo[c2*256 + jo, 1024g + jh*128 + p]
    wos = [
        np.ascontiguousarray(
            Wo[:, 1024 * g : 1024 * (g + 1)]
            .T.reshape(NQH, 128, 8, 256)
            .transpose(1, 2, 0, 3)
            .reshape(128, 8 * 2048)
        ).astype(bf)
        for g in range(2)
    ]

    in_maps = []
    for c in range(8):
        b, g = c // 2, c % 2
        im = {
            "xt": xts[b],
            "wq": wqs[g],
            "wk": wks[g],
            "wv": wvs[g],
            "wo": wos[g],
        }
        im.update(consts)
        in_maps.append(im)

    res = run_bass_kernel_spmd(nc, in_maps, core_ids=list(range(8)), trace=_trace)

    y = np.empty((B, T, C), dtype=np.float32)
    for b in range(B):
        y[b] = (
            res.results[2 * b]["yT"].astype(np.float32) + res.results[2 * b + 1]["yT"].astype(np.float32)
        ).T
    if _trace:
        return y, res
    return y
